# revision 40
# baseline (speedup 1.0000x reference)
"""AttentiveGRU2 message-passing kernel for 8 Trainium2 NeuronCores.

Math (equivalent to the reference; projection moved after aggregation,
edge-softmax weights aw_e = exp(l_e)/esum[dst_e] computed in host prep
alongside the index layouts):

    uT[f,n] = sum_{e: dst=n} aw_e * f_{src_e}[f]   (PE one-hot matmuls)
    cT      = W_proj @ uT;  ctx0 = elu(cT + b_proj) + 1
    GRU(ctx, f) -> relu(h')     (ctx+1 shift folded into gate biases)

Sharding: nodes are range-partitioned over 8 cores (6250 each); each core
owns its incoming edges.  Within a core, nodes are bin-packed (host-side
permutation) into 49 tiles of <=128 so per-tile edge counts are balanced.
Per tile, the segment-sum runs as PE matmuls in FEATURE-major orientation:
    aggT[f, n] += gathered[e, f]^T-contracted-with oh[e, n]
with oh[e, n] = ex_e * [dstloc_e == n] built on DVE in bf16.
Source rows are fetched with dma_gather (int16 indices; edges split into
src<32768 / src>=32768 streams gathered from two table bases) from a bf16
copy of node_feats.
"""

import math
import os

import numpy as np

_SP = int(os.environ.get("K_SP", "0"))     # dma_gather single_packet
_F32G = int(os.environ.get("K_F32G", "0"))  # gather fp32 rows (cheaper Q7/idx)
_GT = int(os.environ.get("K_GT", "2"))     # tiles per gather group
_NQ = int(os.environ.get("K_NQ", "4"))     # SWDGE queues (gather gen parallelism)
_AMOD = int(os.environ.get("K_AMOD", "4"))  # 1/AMOD of oh blocks take the ACT route

P = 128
N_NODES = 50000
N_EDGES = 800000
D = 128
NC = 8
NPC = N_NODES // NC          # 6250 nodes per core
NT = math.ceil(NPC / P)      # 49 node tiles per core
NTP = NT * P                 # 6272 padded node slots per core
HALF = 32768                 # int16 index split point

_nc_cache = {}


def _build_nc(KB_lo, KB_hi):
    import concourse.bacc as bacc
    import concourse.bass as bass
    import concourse.mybir as mybir
    import concourse.tile as tile

    f32 = mybir.dt.float32
    bf16 = mybir.dt.bfloat16
    i16 = mybir.dt.int16
    Alu = mybir.AluOpType
    Act = mybir.ActivationFunctionType

    GT = _GT
    KBT = KB_lo + KB_hi
    NBLK = NT * KBT
    NI_lo = NT * KB_lo * P       # total lo gather slots per core
    NI_hi = NT * KB_hi * P
    NG = math.ceil(NT / GT)      # gather groups / epilogue chunks

    nc = bacc.Bacc(None, target_bir_lowering=False, num_swdge_queues=_NQ)

    gdt = f32 if _F32G else bf16
    nf = nc.dram_tensor("nf", [N_NODES, D], gdt, kind="ExternalInput")
    idx_lo_d = nc.dram_tensor("idx_lo", [P, NI_lo // 16], i16, kind="ExternalInput")
    idx_hi_d = nc.dram_tensor("idx_hi", [P, NI_hi // 16], i16, kind="ExternalInput")
    aw_d = nc.dram_tensor("aw", [P, NBLK], f32, kind="ExternalInput")
    awb_d = nc.dram_tensor("awb", [P, NBLK], bf16, kind="ExternalInput")
    dstlocb_d = nc.dram_tensor("dstlocb", [P, NBLK], bf16, kind="ExternalInput")
    dstneg_d = nc.dram_tensor("dstneg", [P, NBLK], f32, kind="ExternalInput")
    nfT_d = nc.dram_tensor("nfT", [P, NTP], bf16, kind="ExternalInput")
    w_projT_d = nc.dram_tensor("w_projT", [D, D], bf16, kind="ExternalInput")
    w_ihT_d = nc.dram_tensor("w_ihT", [D, 3 * D], bf16, kind="ExternalInput")
    w_hhT_d = nc.dram_tensor("w_hhT", [D, 3 * D], bf16, kind="ExternalInput")
    b_projc_d = nc.dram_tensor("b_projc", [D, 1], f32, kind="ExternalInput")
    b_ih3_d = nc.dram_tensor("b_ih3", [D, 3], f32, kind="ExternalInput")
    b_hh3_d = nc.dram_tensor("b_hh3", [D, 3], f32, kind="ExternalInput")
    iota_d = nc.dram_tensor("iota", [P, P], f32, kind="ExternalInput")
    iotab_d = nc.dram_tensor("iotab", [P, P], bf16, kind="ExternalInput")
    hT_d = nc.dram_tensor("hT", [P, NTP], f32, kind="ExternalOutput")

    with tile.TileContext(nc) as tc:
        with (
            tc.tile_pool(name="const", bufs=1) as cp,
            tc.tile_pool(name="glo_p", bufs=6) as glo_p,
            tc.tile_pool(name="ghi_p", bufs=6) as ghi_p,
            tc.tile_pool(name="oh_p", bufs=10) as oh_p,
            tc.tile_pool(name="u_p", bufs=3) as u_p,
            tc.tile_pool(name="wk", bufs=2) as wk,
            tc.tile_pool(name="ps_agg", bufs=3, space="PSUM") as ps_agg,
            tc.tile_pool(name="ps_ct", bufs=2, space="PSUM") as ps_ct,
            tc.tile_pool(name="ps_misc", bufs=1, space="PSUM") as ps_misc,
            tc.tile_pool(name="ps_g", bufs=1, space="PSUM") as ps_g,
        ):
            # ---- resident tiles -------------------------------------------
            idx_lo = cp.tile([P, NI_lo // 16], i16)
            nc.sync.dma_start(out=idx_lo[:], in_=idx_lo_d[:])
            idx_hi = cp.tile([P, NI_hi // 16], i16)
            nc.sync.dma_start(out=idx_hi[:], in_=idx_hi_d[:])
            aw = cp.tile([P, NBLK], f32)
            nc.sync.dma_start(out=aw[:], in_=aw_d[:])
            awb = cp.tile([P, NBLK], bf16)
            nc.sync.dma_start(out=awb[:], in_=awb_d[:])
            dstlocb = cp.tile([P, NBLK], bf16)
            nc.sync.dma_start(out=dstlocb[:], in_=dstlocb_d[:])
            dstneg = cp.tile([P, NBLK], f32)
            nc.sync.dma_start(out=dstneg[:], in_=dstneg_d[:])
            nfT = cp.tile([P, NTP], bf16)
            nc.sync.dma_start(out=nfT[:], in_=nfT_d[:])
            w_projT = cp.tile([D, D], bf16)
            nc.sync.dma_start(out=w_projT[:], in_=w_projT_d[:])
            w_ihT = cp.tile([D, 3 * D], bf16)
            nc.sync.dma_start(out=w_ihT[:], in_=w_ihT_d[:])
            w_hhT = cp.tile([D, 3 * D], bf16)
            nc.sync.dma_start(out=w_hhT[:], in_=w_hhT_d[:])
            b_projc = cp.tile([D, 1], f32)
            nc.sync.dma_start(out=b_projc[:], in_=b_projc_d[:])
            b_ih3 = cp.tile([D, 3], f32)
            nc.sync.dma_start(out=b_ih3[:], in_=b_ih3_d[:])
            b_hh3 = cp.tile([D, 3], f32)
            nc.sync.dma_start(out=b_hh3[:], in_=b_hh3_d[:])
            iota = cp.tile([P, P], f32)
            nc.sync.dma_start(out=iota[:], in_=iota_d[:])
            iotab = cp.tile([P, P], bf16)
            nc.sync.dma_start(out=iotab[:], in_=iotab_d[:])

            ones_c = cp.tile([P, 1], bf16)
            nc.vector.memset(ones_c[:], 1.0)

            # Warmup gather: pays the gather ucode's one-time ~6us Q7 IRAM
            # load while the real idx tables are still streaming in, so the
            # first real gather starts hot.  Depends only on the memset.
            idxw = cp.tile([P, 8], i16)
            nc.vector.memset(idxw[:], 0)
            gw = cp.tile([P, P], gdt)
            nc.gpsimd.dma_gather(
                out_ap=gw[:, :P].rearrange("p (n e) -> p n e", e=P),
                in_ap=nf[:],
                idxs_ap=idxw[:],
                num_idxs=P,
                num_idxs_reg=P,
                elem_size=P,
                single_packet=bool(_SP),
                queue_num=0,
            )

            # GRU input biases, adjusted for the ctx+1 shift:
            #   ctx0 = elu(c) + 1, so b_ih_g' = b_ih_g - rowsum(W_ih_g).
            rs = ps_misc.tile([P, 3], f32, name="rs", tag="misc")
            for g in range(3):
                nc.tensor.matmul(
                    rs[:, g : g + 1],
                    lhsT=w_ihT[:, g * D : (g + 1) * D],
                    rhs=ones_c[:],
                    start=(g == 0),
                    stop=(g == 2),
                    skip_group_check=True,
                )
            bb = cp.tile([P, 3], f32)
            nc.vector.tensor_add(bb[:, 0:2], b_ih3[:, 0:2], b_hh3[:, 0:2])
            nc.vector.tensor_copy(bb[:, 2:3], b_ih3[:, 2:3])
            nc.vector.tensor_sub(bb[:], bb[:], rs[:])

            # ---- main loop -------------------------------------------------
            # num_idxs registers hoisted out of the loop (one reg per distinct
            # count) so the Pool instruction stream between gathers stays
            # short and the exec queue can run gathers ahead.
            reg_cache = {}

            def idx_reg(n):
                if n not in reg_cache:
                    reg_cache[n] = nc.gpsimd.to_reg(n)
                return reg_cache[n]

            # Each (stream, group) gather is split in two halves; every piece
            # goes to the least-loaded SWDGE queue so the four Q7 core pairs
            # share descriptor generation evenly (lo pieces are ~2x hi ones).
            qloads = [0] * _NQ

            def emit_gather(buf, src_ap, idx_tile, c0_idx, n):
                # buf columns [0, n) filled from idx columns starting c0_idx
                h1 = (n // 2 // P) * P
                for (off, cnt) in ((0, h1), (h1, n - h1)):
                    if cnt == 0:
                        continue
                    q = min(range(_NQ), key=lambda i: qloads[i])
                    qloads[q] += cnt
                    nc.gpsimd.dma_gather(
                        out_ap=buf[:, off : off + cnt].rearrange(
                            "p (n e) -> p n e", e=P),
                        in_ap=src_ap,
                        idxs_ap=idx_tile[:, (c0_idx + off) // 16 :
                                         (c0_idx + off + cnt) // 16],
                        num_idxs=cnt,
                        num_idxs_reg=idx_reg(cnt),
                        elem_size=P,
                        single_packet=bool(_SP),
                        queue_num=q,
                    )

            for g in range(NG):
                t0 = g * GT
                t1 = min(t0 + GT, NT)
                nt = t1 - t0
                n_lo = nt * KB_lo * P
                n_hi = nt * KB_hi * P

                glo = glo_p.tile([P, GT * KB_lo * P], gdt, name="glo")
                emit_gather(glo, nf[:], idx_lo, t0 * KB_lo * P, n_lo)
                ghi = ghi_p.tile([P, GT * KB_hi * P], gdt, name="ghi")
                emit_gather(ghi, nf[HALF:, :], idx_hi, t0 * KB_hi * P, n_hi)

                uch = u_p.tile([P, GT * P], bf16, name="uch")
                for t in range(t0, t1):
                    ps = ps_agg.tile([P, P], f32, name="ps")
                    for j in range(KBT):
                        col = t * KBT + j
                        if j < KB_lo:
                            lhs = glo[:, ((t - t0) * KB_lo + j) * P :][:, :P]
                        else:
                            jj = j - KB_lo
                            lhs = ghi[:, ((t - t0) * KB_hi + jj) * P :][:, :P]
                        oh = oh_p.tile([P, P], gdt, name="oh")
                        if (t * KBT + j) % _AMOD == _AMOD - 1:
                            # ACT route (bf16 for the 2x perf mode):
                            #   oh = relu(aw - 1e6*(iota - dst)^2)
                            # (iota-dst)^2 <= 127^2 rounds in bf16 but stays
                            # nonzero, which is all the -1e6 mask needs.
                            sq = oh_p.tile([P, P], bf16, name="sq", tag="sq")
                            nc.scalar.activation(sq[:], iotab[:], Act.Square,
                                                 bias=dstneg[:, col : col + 1])
                            nc.scalar.activation(oh[:], sq[:], Act.Relu,
                                                 scale=-1e6,
                                                 bias=aw[:, col : col + 1])
                        else:
                            # DVE route, one bf16 STT op with the weight
                            # column broadcast (0-stride) along free:
                            #   oh = (iota == dst) * aw_bc
                            nc.vector.scalar_tensor_tensor(
                                oh[:], iotab[:],
                                dstlocb[:, col : col + 1],
                                awb[:, col : col + 1].to_broadcast((P, P)),
                                Alu.is_equal, Alu.mult,
                            )
                        nc.tensor.matmul(
                            ps[:, 0:P], lhsT=lhs, rhs=oh[:],
                            start=(j == 0), stop=(j == KBT - 1),
                            skip_group_check=True,
                        )
                    # softmax weights are pre-normalized; just move the
                    # accumulated tile PSUM -> SBUF for the projection GEMM
                    nc.vector.tensor_copy(
                        uch[:, (t - t0) * P : (t - t0 + 1) * P], ps[:, 0:P],
                    )

                # ---- chunk epilogue (nodes t0*P .. t1*P) ------------------
                w = nt * P
                fch = nfT[:, t0 * P : t0 * P + w]

                ct = ps_ct.tile([P, GT * P], f32, name="ct", tag="ct")
                nc.tensor.matmul(ct[:, :w], lhsT=w_projT[:], rhs=uch[:, :w],
                                 start=True, stop=True, skip_group_check=True)
                # ctx0 = elu(c + b_proj) + 1 = relu(c+b) + min(exp(c+b), 1)
                expc = wk.tile([P, GT * P], bf16, name="expc")
                nc.scalar.activation(expc[:, :w], ct[:, :w], Act.Exp,
                                     bias=b_projc[:, 0:1])
                reluc = wk.tile([P, GT * P], bf16, name="reluc")
                nc.scalar.activation(reluc[:, :w], ct[:, :w], Act.Relu,
                                     bias=b_projc[:, 0:1])
                ctx0 = wk.tile([P, GT * P], bf16, name="ctx0")
                nc.vector.scalar_tensor_tensor(
                    ctx0[:, :w], expc[:, :w], 1.0, reluc[:, :w],
                    Alu.min, Alu.add,
                )

                # GRU gates at fixed GT*P strides: r | z | in_ | hn
                # (bank A holds r,z; bank B holds in_,hn; the start=True of
                # each bank's first matmul zeroes that whole bank).
                GP = GT * P
                pg = ps_g.tile([P, 4 * GP], f32, name="pg")
                nc.tensor.matmul(pg[:, 0 * GP :][:, :w], lhsT=w_ihT[:, 0:D],
                                 rhs=ctx0[:, :w], start=True, stop=False,
                                 skip_group_check=True)
                nc.tensor.matmul(pg[:, 0 * GP :][:, :w], lhsT=w_hhT[:, 0:D],
                                 rhs=fch, start=False, stop=True,
                                 skip_group_check=True)
                nc.tensor.matmul(pg[:, 1 * GP :][:, :w], lhsT=w_ihT[:, D : 2 * D],
                                 rhs=ctx0[:, :w], start=False, stop=False,
                                 skip_group_check=True)
                nc.tensor.matmul(pg[:, 1 * GP :][:, :w], lhsT=w_hhT[:, D : 2 * D],
                                 rhs=fch, start=False, stop=True,
                                 skip_group_check=True)
                nc.tensor.matmul(pg[:, 2 * GP :][:, :w], lhsT=w_ihT[:, 2 * D : 3 * D],
                                 rhs=ctx0[:, :w], start=True, stop=True,
                                 skip_group_check=True)
                nc.tensor.matmul(pg[:, 3 * GP :][:, :w], lhsT=w_hhT[:, 2 * D : 3 * D],
                                 rhs=fch, start=False, stop=True,
                                 skip_group_check=True)

                rT = wk.tile([P, GT * P], f32, name="rT")
                nc.scalar.activation(rT[:, :w], pg[:, 0 * GP :][:, :w], Act.Sigmoid,
                                     bias=bb[:, 0:1])
                zT = wk.tile([P, GT * P], bf16, name="zT")
                nc.scalar.activation(zT[:, :w], pg[:, 1 * GP :][:, :w], Act.Sigmoid,
                                     bias=bb[:, 1:2])
                # rhn = (phn + b_hh_n) * r
                rhn = wk.tile([P, GT * P], f32, name="rhn")
                nc.vector.scalar_tensor_tensor(
                    rhn[:, :w], pg[:, 3 * GP :][:, :w], b_hh3[:, 2:3], rT[:, :w],
                    Alu.add, Alu.mult,
                )
                tn = wk.tile([P, GT * P], f32, name="tn")
                nc.vector.tensor_add(tn[:, :w], pg[:, 2 * GP :][:, :w], rhn[:, :w])
                nT = wk.tile([P, GT * P], bf16, name="nT")
                nc.scalar.activation(nT[:, :w], tn[:, :w], Act.Tanh,
                                     bias=bb[:, 2:3])
                # h = n + z*(f - n); out = relu(h)
                d2 = wk.tile([P, GT * P], bf16, name="d2")
                nc.vector.tensor_sub(d2[:, :w], fch, nT[:, :w])
                e2 = wk.tile([P, GT * P], bf16, name="e2")
                nc.vector.tensor_mul(e2[:, :w], zT[:, :w], d2[:, :w])
                h1 = wk.tile([P, GT * P], bf16, name="h1")
                nc.vector.tensor_add(h1[:, :w], nT[:, :w], e2[:, :w])
                hch = wk.tile([P, GT * P], f32, name="hch")
                nc.scalar.activation(hch[:, :w], h1[:, :w], Act.Relu)
                nc.sync.dma_start(out=hT_d[:, t0 * P : t0 * P + w],
                                  in_=hch[:, :w])

    nc.compile()
    return nc


def _balance_tiles(lo_deg, hi_deg, kb_lo_t, kb_hi_t):
    """Pack NPC nodes into NT bins of <=P nodes, balancing lo/hi edge sums.

    Returns slot[NPC] -> global slot id (tile*P + lane).  Greedy: nodes in
    descending total degree, placed in the feasible bin minimizing the max
    normalized load.
    """
    cap_lo = kb_lo_t * P
    cap_hi = kb_hi_t * P
    order = np.argsort(-(lo_deg + hi_deg), kind="stable")
    bin_lo = np.zeros(NT, np.int64)
    bin_hi = np.zeros(NT, np.int64)
    bin_cnt = np.zeros(NT, np.int64)
    slot = np.zeros(NPC, np.int64)
    for v in order:
        load = np.maximum((bin_lo + lo_deg[v]) / cap_lo,
                          (bin_hi + hi_deg[v]) / cap_hi)
        load[bin_cnt >= P] = np.inf
        b = int(np.argmin(load))
        slot[v] = b * P + bin_cnt[b]
        bin_cnt[b] += 1
        bin_lo[b] += lo_deg[v]
        bin_hi[b] += hi_deg[v]
    return slot, bin_lo, bin_hi


def _prep_inputs(edge_logits, node_feats, src, dst, W_proj, b_proj, W_ih, b_ih,
                 W_hh, b_hh):
    """Host-side index preprocessing + layout. Returns (in_maps, KB_lo, KB_hi, slots)."""
    import ml_dtypes

    bf16 = ml_dtypes.bfloat16
    src = np.asarray(src).astype(np.int64)
    dst = np.asarray(dst).astype(np.int64)
    logit = np.asarray(edge_logits, dtype=np.float32).reshape(-1)
    nf = np.ascontiguousarray(np.asarray(node_feats, dtype=np.float32))

    core = dst // NPC
    loc = dst - core * NPC
    is_lo = src < HALF

    # per-(core, local node) in-degrees for each stream
    key = core * NPC + loc
    lo_deg = np.bincount(key[is_lo], minlength=NC * NPC).reshape(NC, NPC)
    hi_deg = np.bincount(key[~is_lo], minlength=NC * NPC).reshape(NC, NPC)
    tot_deg = lo_deg + hi_deg
    assert tot_deg.max() <= P, f"in-degree {tot_deg.max()} exceeds {P}"
    assert (lo_deg + hi_deg).reshape(-1).min() >= 0

    # balance nodes into tiles; then KB = max blocks over all (core, tile)
    kb_lo_t = max(1, int(math.ceil(lo_deg.sum(1).max() / (NT * P))))
    kb_hi_t = max(1, int(math.ceil(hi_deg.sum(1).max() / (NT * P))))
    slots = np.zeros((NC, NPC), np.int64)
    KB_lo = kb_lo_t
    KB_hi = kb_hi_t
    for c in range(NC):
        s, bl, bh = _balance_tiles(lo_deg[c], hi_deg[c], kb_lo_t, kb_hi_t)
        slots[c] = s
        KB_lo = max(KB_lo, int(math.ceil(bl.max() / P)))
        KB_hi = max(KB_hi, int(math.ceil(bh.max() / P)))
    KBT = KB_lo + KB_hi
    NBLK = NT * KBT
    NI_lo = NT * KB_lo * P
    NI_hi = NT * KB_hi * P

    # per-edge slot position
    eslot = slots[core, loc]          # global slot id of dst
    tl = eslot >> 7                   # tile
    lane = eslot & 127                # node lane within tile

    # host edge-softmax (same math as the reference, shift-free exp since
    # logits are O(1)): aw_e = exp(l_e) / sum_{e': dst=dst_e} exp(l_e')
    ex_h = np.exp(logit.astype(np.float64))
    esum = np.bincount(dst, weights=ex_h, minlength=N_NODES)
    aw_e = (ex_h / esum[dst]).astype(np.float32)

    aw_arr = np.zeros((NC, P, NBLK), np.float32)
    dl_arr = np.zeros((NC, P, NBLK), np.float32)
    # Pad slots hold idx -1: the gather ucode trims trailing negatives at
    # runtime (per core), skipping their descriptor generation AND the DMA.
    # Skipped slots keep stale SBUF data, harmless since their one-hot
    # column is all-zero -- except on a buffer's FIRST use (first 6 groups,
    # uninitialized SBUF could be NaN as bf16), where pads gather row 0.
    ilo_flat = np.full((NC, NI_lo), -1, np.int16)
    ihi_flat = np.full((NC, NI_hi), -1, np.int16)

    tkey = core * NT + tl
    for stream, KB, ifl, coff in ((is_lo, KB_lo, ilo_flat, 0),
                                  (~is_lo, KB_hi, ihi_flat, KB_lo)):
        sel = np.flatnonzero(stream)
        order = sel[np.argsort(tkey[sel], kind="stable")]
        cnts = np.bincount(tkey[sel], minlength=NC * NT)
        assert cnts.max() <= KB * P
        starts = np.zeros(NC * NT, np.int64)
        starts[1:] = np.cumsum(cnts)[:-1]
        rank = np.arange(order.size, dtype=np.int64) - starts[tkey[order]]
        j = rank >> 7
        p = rank & 127
        ce = core[order]
        te = tl[order]
        col = te * KBT + coff + j
        aw_arr[ce, p, col] = aw_e[order]
        dl_arr[ce, p, col] = lane[order].astype(np.float32)
        sv = src[order] - (0 if coff == 0 else HALF)
        ifl[ce, (te * KB + j) * P + p] = sv.astype(np.int16)
        # first-use buffers (first GT*6 tiles, ring of 6): pads gather row 0
        first = 6 * _GT * KB * P
        ifl[:, :first][ifl[:, :first] < 0] = 0

    def wrap16(flat):  # [NC, L] -> [NC, 128, L//16]
        L = flat.shape[1]
        w = flat.reshape(NC, L // 16, 16).transpose(0, 2, 1)  # [NC,16,L//16]
        return np.ascontiguousarray(np.tile(w, (1, 8, 1)))

    ilo = wrap16(ilo_flat)
    ihi = wrap16(ihi_flat)

    nfT = np.zeros((NC, P, NTP), bf16)
    inv_slots = np.zeros((NC, NPC), np.int64)
    for c in range(NC):
        nfT[c][:, slots[c]] = nf[c * NPC : (c + 1) * NPC].T
        inv_slots[c] = slots[c]

    shared = {
        "nf": nf if _F32G else nf.astype(bf16),
        "w_projT": np.ascontiguousarray(np.asarray(W_proj, np.float32).T).astype(bf16),
        "w_ihT": np.ascontiguousarray(np.asarray(W_ih, np.float32).T).astype(bf16),
        "w_hhT": np.ascontiguousarray(np.asarray(W_hh, np.float32).T).astype(bf16),
        "b_projc": np.asarray(b_proj, np.float32).reshape(D, 1),
        "b_ih3": np.ascontiguousarray(np.asarray(b_ih, np.float32).reshape(3, D).T),
        "b_hh3": np.ascontiguousarray(np.asarray(b_hh, np.float32).reshape(3, D).T),
        "iota": np.ascontiguousarray(
            np.broadcast_to(np.arange(P, dtype=np.float32), (P, P))),
        "iotab": np.ascontiguousarray(
            np.broadcast_to(np.arange(P, dtype=np.float32), (P, P))).astype(bf16),
    }
    in_maps = []
    for c in range(NC):
        m = dict(shared)
        m["idx_lo"] = ilo[c]
        m["idx_hi"] = ihi[c]
        m["aw"] = aw_arr[c]
        m["awb"] = aw_arr[c].astype(bf16)
        m["dstlocb"] = dl_arr[c].astype(bf16)
        m["dstneg"] = -dl_arr[c]
        m["nfT"] = nfT[c]
        in_maps.append(m)
    return in_maps, KB_lo, KB_hi, inv_slots


def _run(inputs, trace=False):
    from concourse.bass_utils import run_bass_kernel_spmd

    in_maps, KB_lo, KB_hi, slots = _prep_inputs(**inputs)
    key = (KB_lo, KB_hi, _SP, _GT, _F32G, _NQ, _AMOD)
    if key not in _nc_cache:
        _nc_cache[key] = _build_nc(KB_lo, KB_hi)
    nc = _nc_cache[key]
    res = run_bass_kernel_spmd(nc, in_maps, core_ids=list(range(NC)), trace=trace)
    out = np.empty((N_NODES, D), np.float32)
    for c in range(NC):
        hT = res.results[c]["hT"]
        out[c * NPC : (c + 1) * NPC] = hT[:, slots[c]].T
    return out, res


def kernel(**inputs):
    out, _ = _run(inputs, trace=False)
    return out



# revision 43
# speedup vs baseline: 1.0000x; 1.0000x over previous
"""AttentiveGRU2 message-passing kernel for 8 Trainium2 NeuronCores.

Math (equivalent to the reference; projection moved after aggregation,
edge-softmax weights aw_e = exp(l_e)/esum[dst_e] computed in host prep
alongside the index layouts):

    uT[f,n] = sum_{e: dst=n} aw_e * f_{src_e}[f]   (PE one-hot matmuls)
    cT      = W_proj @ uT;  ctx0 = elu(cT + b_proj) + 1
    GRU(ctx, f) -> relu(h')     (ctx+1 shift folded into gate biases)

Sharding: nodes are range-partitioned over 8 cores (6250 each); each core
owns its incoming edges.  Within a core, nodes are bin-packed (host-side
permutation) into 49 tiles of <=128 so per-tile edge counts are balanced.
Per tile, the segment-sum runs as PE matmuls in FEATURE-major orientation:
    aggT[f, n] += gathered[e, f]^T-contracted-with oh[e, n]
with oh[e, n] = ex_e * [dstloc_e == n] built on DVE in bf16.
Source rows are fetched with dma_gather (int16 indices; edges split into
src<32768 / src>=32768 streams gathered from two table bases) from a bf16
copy of node_feats.
"""

import math
import os

import numpy as np

_SP = int(os.environ.get("K_SP", "0"))     # dma_gather single_packet
_F32G = int(os.environ.get("K_F32G", "0"))  # gather fp32 rows (cheaper Q7/idx)
_GT = int(os.environ.get("K_GT", "2"))     # tiles per gather group
_NQ = int(os.environ.get("K_NQ", "4"))     # SWDGE queues (gather gen parallelism)
_AMOD = int(os.environ.get("K_AMOD", "4"))  # 1/AMOD of oh blocks take the ACT route

P = 128
N_NODES = 50000
N_EDGES = 800000
D = 128
NC = 8
NPC = N_NODES // NC          # 6250 nodes per core
NT = math.ceil(NPC / P)      # 49 node tiles per core
NTP = NT * P                 # 6272 padded node slots per core
HALF = 32768                 # int16 index split point

_nc_cache = {}


def _build_nc(KB_lo, KB_hi):
    import concourse.bacc as bacc
    import concourse.bass as bass
    import concourse.mybir as mybir
    import concourse.tile as tile

    f32 = mybir.dt.float32
    bf16 = mybir.dt.bfloat16
    i16 = mybir.dt.int16
    Alu = mybir.AluOpType
    Act = mybir.ActivationFunctionType

    GT = _GT
    KBT = KB_lo + KB_hi
    NBLK = NT * KBT
    NI_lo = NT * KB_lo * P       # total lo gather slots per core
    NI_hi = NT * KB_hi * P
    NG = math.ceil(NT / GT)      # gather groups / epilogue chunks

    nc = bacc.Bacc(None, target_bir_lowering=False, num_swdge_queues=_NQ)

    gdt = f32 if _F32G else bf16
    nf = nc.dram_tensor("nf", [N_NODES, D], gdt, kind="ExternalInput")
    idx_lo_d = nc.dram_tensor("idx_lo", [P, NI_lo // 16], i16, kind="ExternalInput")
    idx_hi_d = nc.dram_tensor("idx_hi", [P, NI_hi // 16], i16, kind="ExternalInput")
    aw_d = nc.dram_tensor("aw", [P, NBLK], f32, kind="ExternalInput")
    awb_d = nc.dram_tensor("awb", [P, NBLK], bf16, kind="ExternalInput")
    dstlocb_d = nc.dram_tensor("dstlocb", [P, NBLK], bf16, kind="ExternalInput")
    dstneg_d = nc.dram_tensor("dstneg", [P, NBLK], f32, kind="ExternalInput")
    nfT_d = nc.dram_tensor("nfT", [P, NTP], bf16, kind="ExternalInput")
    w_projT_d = nc.dram_tensor("w_projT", [D, D], bf16, kind="ExternalInput")
    w_ihT_d = nc.dram_tensor("w_ihT", [D, 3 * D], bf16, kind="ExternalInput")
    w_hhT_d = nc.dram_tensor("w_hhT", [D, 3 * D], bf16, kind="ExternalInput")
    b_projc_d = nc.dram_tensor("b_projc", [D, 1], f32, kind="ExternalInput")
    b_ih3_d = nc.dram_tensor("b_ih3", [D, 3], f32, kind="ExternalInput")
    b_hh3_d = nc.dram_tensor("b_hh3", [D, 3], f32, kind="ExternalInput")
    iota_d = nc.dram_tensor("iota", [P, P], f32, kind="ExternalInput")
    iotab_d = nc.dram_tensor("iotab", [P, P], bf16, kind="ExternalInput")
    hT_d = nc.dram_tensor("hT", [P, NTP], f32, kind="ExternalOutput")

    with tile.TileContext(nc) as tc:
        with (
            tc.tile_pool(name="const", bufs=1) as cp,
            tc.tile_pool(name="glo_p", bufs=8) as glo_p,
            tc.tile_pool(name="ghi_p", bufs=8) as ghi_p,
            tc.tile_pool(name="oh_p", bufs=10) as oh_p,
            tc.tile_pool(name="u_p", bufs=3) as u_p,
            tc.tile_pool(name="wk", bufs=2) as wk,
            tc.tile_pool(name="ps_agg", bufs=3, space="PSUM") as ps_agg,
            tc.tile_pool(name="ps_ct", bufs=2, space="PSUM") as ps_ct,
            tc.tile_pool(name="ps_misc", bufs=1, space="PSUM") as ps_misc,
            tc.tile_pool(name="ps_g", bufs=1, space="PSUM") as ps_g,
        ):
            # ---- resident tiles -------------------------------------------
            # idx tables split into a small head (first 2 groups) + tail so
            # the first gathers only wait for a tiny DMA, not the whole table
            LOH = 2 * GT * KB_lo * P // 16   # head cols (idx/16)
            HIH = 2 * GT * KB_hi * P // 16
            idx_lo_a = cp.tile([P, LOH], i16)
            nc.sync.dma_start(out=idx_lo_a[:], in_=idx_lo_d[:, :LOH])
            idx_hi_a = cp.tile([P, HIH], i16)
            nc.sync.dma_start(out=idx_hi_a[:], in_=idx_hi_d[:, :HIH])
            idx_lo_b = cp.tile([P, NI_lo // 16 - LOH], i16)
            nc.sync.dma_start(out=idx_lo_b[:], in_=idx_lo_d[:, LOH:])
            idx_hi_b = cp.tile([P, NI_hi // 16 - HIH], i16)
            nc.sync.dma_start(out=idx_hi_b[:], in_=idx_hi_d[:, HIH:])
            aw = cp.tile([P, NBLK], f32)
            nc.sync.dma_start(out=aw[:], in_=aw_d[:])
            awb = cp.tile([P, NBLK], bf16)
            nc.sync.dma_start(out=awb[:], in_=awb_d[:])
            dstlocb = cp.tile([P, NBLK], bf16)
            nc.sync.dma_start(out=dstlocb[:], in_=dstlocb_d[:])
            dstneg = cp.tile([P, NBLK], f32)
            nc.sync.dma_start(out=dstneg[:], in_=dstneg_d[:])
            nfT = cp.tile([P, NTP], bf16)
            nc.sync.dma_start(out=nfT[:], in_=nfT_d[:])
            w_projT = cp.tile([D, D], bf16)
            nc.sync.dma_start(out=w_projT[:], in_=w_projT_d[:])
            w_ihT = cp.tile([D, 3 * D], bf16)
            nc.sync.dma_start(out=w_ihT[:], in_=w_ihT_d[:])
            w_hhT = cp.tile([D, 3 * D], bf16)
            nc.sync.dma_start(out=w_hhT[:], in_=w_hhT_d[:])
            b_projc = cp.tile([D, 1], f32)
            nc.sync.dma_start(out=b_projc[:], in_=b_projc_d[:])
            b_ih3 = cp.tile([D, 3], f32)
            nc.sync.dma_start(out=b_ih3[:], in_=b_ih3_d[:])
            b_hh3 = cp.tile([D, 3], f32)
            nc.sync.dma_start(out=b_hh3[:], in_=b_hh3_d[:])
            iota = cp.tile([P, P], f32)
            nc.sync.dma_start(out=iota[:], in_=iota_d[:])
            iotab = cp.tile([P, P], bf16)
            nc.sync.dma_start(out=iotab[:], in_=iotab_d[:])

            ones_c = cp.tile([P, 1], bf16)
            nc.vector.memset(ones_c[:], 1.0)

            # Warmup gather: pays the gather ucode's one-time ~6us Q7 IRAM
            # load while the real idx tables are still streaming in, so the
            # first real gather starts hot.  Depends only on the memset.
            idxw = cp.tile([P, 8], i16)
            nc.vector.memset(idxw[:], 0)
            gw = cp.tile([P, P], gdt)
            nc.gpsimd.dma_gather(
                out_ap=gw[:, :P].rearrange("p (n e) -> p n e", e=P),
                in_ap=nf[:],
                idxs_ap=idxw[:],
                num_idxs=P,
                num_idxs_reg=P,
                elem_size=P,
                single_packet=bool(_SP),
                queue_num=0,
            )

            # GRU input biases, adjusted for the ctx+1 shift:
            #   ctx0 = elu(c) + 1, so b_ih_g' = b_ih_g - rowsum(W_ih_g).
            rs = ps_misc.tile([P, 3], f32, name="rs", tag="misc")
            for g in range(3):
                nc.tensor.matmul(
                    rs[:, g : g + 1],
                    lhsT=w_ihT[:, g * D : (g + 1) * D],
                    rhs=ones_c[:],
                    start=(g == 0),
                    stop=(g == 2),
                    skip_group_check=True,
                )
            bb = cp.tile([P, 3], f32)
            nc.vector.tensor_add(bb[:, 0:2], b_ih3[:, 0:2], b_hh3[:, 0:2])
            nc.vector.tensor_copy(bb[:, 2:3], b_ih3[:, 2:3])
            nc.vector.tensor_sub(bb[:], bb[:], rs[:])

            # ---- main loop -------------------------------------------------
            # num_idxs registers hoisted out of the loop (one reg per distinct
            # count) so the Pool instruction stream between gathers stays
            # short and the exec queue can run gathers ahead.
            reg_cache = {}

            def idx_reg(n):
                if n not in reg_cache:
                    reg_cache[n] = nc.gpsimd.to_reg(n)
                return reg_cache[n]

            # Each (stream, group) gather is split in two halves; every piece
            # goes to the least-loaded SWDGE queue so the four Q7 core pairs
            # share descriptor generation evenly (lo pieces are ~2x hi ones).
            qloads = [0] * _NQ

            def emit_gather(buf, src_ap, idx_tile, c0_idx, n):
                # buf columns [0, n) filled from idx columns starting c0_idx
                h1 = (n // 2 // P) * P
                for (off, cnt) in ((0, h1), (h1, n - h1)):
                    if cnt == 0:
                        continue
                    q = min(range(_NQ), key=lambda i: qloads[i])
                    qloads[q] += cnt
                    nc.gpsimd.dma_gather(
                        out_ap=buf[:, off : off + cnt].rearrange(
                            "p (n e) -> p n e", e=P),
                        in_ap=src_ap,
                        idxs_ap=idx_tile[:, (c0_idx + off) // 16 :
                                         (c0_idx + off + cnt) // 16],
                        num_idxs=cnt,
                        num_idxs_reg=idx_reg(cnt),
                        elem_size=P,
                        single_packet=bool(_SP),
                        queue_num=q,
                    )

            for g in range(NG):
                t0 = g * GT
                t1 = min(t0 + GT, NT)
                nt = t1 - t0
                n_lo = nt * KB_lo * P
                n_hi = nt * KB_hi * P

                lo_head = 2 * GT * KB_lo * P
                hi_head = 2 * GT * KB_hi * P
                glo = glo_p.tile([P, GT * KB_lo * P], gdt, name="glo")
                if g < 2:
                    emit_gather(glo, nf[:], idx_lo_a, t0 * KB_lo * P, n_lo)
                else:
                    emit_gather(glo, nf[:], idx_lo_b,
                                t0 * KB_lo * P - lo_head, n_lo)
                ghi = ghi_p.tile([P, GT * KB_hi * P], gdt, name="ghi")
                if g < 2:
                    emit_gather(ghi, nf[HALF:, :], idx_hi_a, t0 * KB_hi * P, n_hi)
                else:
                    emit_gather(ghi, nf[HALF:, :], idx_hi_b,
                                t0 * KB_hi * P - hi_head, n_hi)

                uch = u_p.tile([P, GT * P], bf16, name="uch")
                for t in range(t0, t1):
                    ps = ps_agg.tile([P, P], f32, name="ps")
                    for j in range(KBT):
                        col = t * KBT + j
                        if j < KB_lo:
                            lhs = glo[:, ((t - t0) * KB_lo + j) * P :][:, :P]
                        else:
                            jj = j - KB_lo
                            lhs = ghi[:, ((t - t0) * KB_hi + jj) * P :][:, :P]
                        oh = oh_p.tile([P, P], gdt, name="oh")
                        if (t * KBT + j) % _AMOD == _AMOD - 1:
                            # ACT route (bf16 for the 2x perf mode):
                            #   oh = relu(aw - 1e6*(iota - dst)^2)
                            # (iota-dst)^2 <= 127^2 rounds in bf16 but stays
                            # nonzero, which is all the -1e6 mask needs.
                            sq = oh_p.tile([P, P], bf16, name="sq", tag="sq")
                            nc.scalar.activation(sq[:], iotab[:], Act.Square,
                                                 bias=dstneg[:, col : col + 1])
                            nc.scalar.activation(oh[:], sq[:], Act.Relu,
                                                 scale=-1e6,
                                                 bias=aw[:, col : col + 1])
                        else:
                            # DVE route, one bf16 STT op with the weight
                            # column broadcast (0-stride) along free:
                            #   oh = (iota == dst) * aw_bc
                            nc.vector.scalar_tensor_tensor(
                                oh[:], iotab[:],
                                dstlocb[:, col : col + 1],
                                awb[:, col : col + 1].to_broadcast((P, P)),
                                Alu.is_equal, Alu.mult,
                            )
                        nc.tensor.matmul(
                            ps[:, 0:P], lhsT=lhs, rhs=oh[:],
                            start=(j == 0), stop=(j == KBT - 1),
                            skip_group_check=True,
                        )
                    # softmax weights are pre-normalized; just move the
                    # accumulated tile PSUM -> SBUF for the projection GEMM
                    nc.vector.tensor_copy(
                        uch[:, (t - t0) * P : (t - t0 + 1) * P], ps[:, 0:P],
                    )

                # ---- chunk epilogue (nodes t0*P .. t1*P) ------------------
                w = nt * P
                fch = nfT[:, t0 * P : t0 * P + w]

                ct = ps_ct.tile([P, GT * P], f32, name="ct", tag="ct")
                nc.tensor.matmul(ct[:, :w], lhsT=w_projT[:], rhs=uch[:, :w],
                                 start=True, stop=True, skip_group_check=True)
                # ctx0 = elu(c + b_proj) + 1 = relu(c+b) + min(exp(c+b), 1)
                expc = wk.tile([P, GT * P], bf16, name="expc")
                nc.scalar.activation(expc[:, :w], ct[:, :w], Act.Exp,
                                     bias=b_projc[:, 0:1])
                reluc = wk.tile([P, GT * P], bf16, name="reluc")
                nc.scalar.activation(reluc[:, :w], ct[:, :w], Act.Relu,
                                     bias=b_projc[:, 0:1])
                ctx0 = wk.tile([P, GT * P], bf16, name="ctx0")
                nc.vector.scalar_tensor_tensor(
                    ctx0[:, :w], expc[:, :w], 1.0, reluc[:, :w],
                    Alu.min, Alu.add,
                )

                # GRU gates at fixed GT*P strides: r | z | in_ | hn
                # (bank A holds r,z; bank B holds in_,hn; the start=True of
                # each bank's first matmul zeroes that whole bank).
                GP = GT * P
                pg = ps_g.tile([P, 4 * GP], f32, name="pg")
                nc.tensor.matmul(pg[:, 0 * GP :][:, :w], lhsT=w_ihT[:, 0:D],
                                 rhs=ctx0[:, :w], start=True, stop=False,
                                 skip_group_check=True)
                nc.tensor.matmul(pg[:, 0 * GP :][:, :w], lhsT=w_hhT[:, 0:D],
                                 rhs=fch, start=False, stop=True,
                                 skip_group_check=True)
                nc.tensor.matmul(pg[:, 1 * GP :][:, :w], lhsT=w_ihT[:, D : 2 * D],
                                 rhs=ctx0[:, :w], start=False, stop=False,
                                 skip_group_check=True)
                nc.tensor.matmul(pg[:, 1 * GP :][:, :w], lhsT=w_hhT[:, D : 2 * D],
                                 rhs=fch, start=False, stop=True,
                                 skip_group_check=True)
                nc.tensor.matmul(pg[:, 2 * GP :][:, :w], lhsT=w_ihT[:, 2 * D : 3 * D],
                                 rhs=ctx0[:, :w], start=True, stop=True,
                                 skip_group_check=True)
                nc.tensor.matmul(pg[:, 3 * GP :][:, :w], lhsT=w_hhT[:, 2 * D : 3 * D],
                                 rhs=fch, start=False, stop=True,
                                 skip_group_check=True)

                rT = wk.tile([P, GT * P], f32, name="rT")
                nc.scalar.activation(rT[:, :w], pg[:, 0 * GP :][:, :w], Act.Sigmoid,
                                     bias=bb[:, 0:1])
                zT = wk.tile([P, GT * P], bf16, name="zT")
                nc.scalar.activation(zT[:, :w], pg[:, 1 * GP :][:, :w], Act.Sigmoid,
                                     bias=bb[:, 1:2])
                # rhn = (phn + b_hh_n) * r
                rhn = wk.tile([P, GT * P], f32, name="rhn")
                nc.vector.scalar_tensor_tensor(
                    rhn[:, :w], pg[:, 3 * GP :][:, :w], b_hh3[:, 2:3], rT[:, :w],
                    Alu.add, Alu.mult,
                )
                tn = wk.tile([P, GT * P], f32, name="tn")
                nc.vector.tensor_add(tn[:, :w], pg[:, 2 * GP :][:, :w], rhn[:, :w])
                nT = wk.tile([P, GT * P], bf16, name="nT")
                nc.scalar.activation(nT[:, :w], tn[:, :w], Act.Tanh,
                                     bias=bb[:, 2:3])
                # h = n + z*(f - n); out = relu(h)
                d2 = wk.tile([P, GT * P], bf16, name="d2")
                nc.vector.tensor_sub(d2[:, :w], fch, nT[:, :w])
                e2 = wk.tile([P, GT * P], bf16, name="e2")
                nc.vector.tensor_mul(e2[:, :w], zT[:, :w], d2[:, :w])
                h1 = wk.tile([P, GT * P], bf16, name="h1")
                nc.vector.tensor_add(h1[:, :w], nT[:, :w], e2[:, :w])
                hch = wk.tile([P, GT * P], f32, name="hch")
                nc.scalar.activation(hch[:, :w], h1[:, :w], Act.Relu)
                nc.sync.dma_start(out=hT_d[:, t0 * P : t0 * P + w],
                                  in_=hch[:, :w])

    nc.compile()
    return nc


def _balance_tiles(lo_deg, hi_deg, kb_lo_t, kb_hi_t):
    """Pack NPC nodes into NT bins of <=P nodes, balancing lo/hi edge sums.

    Returns slot[NPC] -> global slot id (tile*P + lane).  Greedy: nodes in
    descending total degree, placed in the feasible bin minimizing the max
    normalized load.
    """
    cap_lo = kb_lo_t * P
    cap_hi = kb_hi_t * P
    order = np.argsort(-(lo_deg + hi_deg), kind="stable")
    bin_lo = np.zeros(NT, np.int64)
    bin_hi = np.zeros(NT, np.int64)
    bin_cnt = np.zeros(NT, np.int64)
    slot = np.zeros(NPC, np.int64)
    for v in order:
        load = np.maximum((bin_lo + lo_deg[v]) / cap_lo,
                          (bin_hi + hi_deg[v]) / cap_hi)
        load[bin_cnt >= P] = np.inf
        b = int(np.argmin(load))
        slot[v] = b * P + bin_cnt[b]
        bin_cnt[b] += 1
        bin_lo[b] += lo_deg[v]
        bin_hi[b] += hi_deg[v]
    return slot, bin_lo, bin_hi


def _prep_inputs(edge_logits, node_feats, src, dst, W_proj, b_proj, W_ih, b_ih,
                 W_hh, b_hh):
    """Host-side index preprocessing + layout. Returns (in_maps, KB_lo, KB_hi, slots)."""
    import ml_dtypes

    bf16 = ml_dtypes.bfloat16
    src = np.asarray(src).astype(np.int64)
    dst = np.asarray(dst).astype(np.int64)
    logit = np.asarray(edge_logits, dtype=np.float32).reshape(-1)
    nf = np.ascontiguousarray(np.asarray(node_feats, dtype=np.float32))

    core = dst // NPC
    loc = dst - core * NPC
    is_lo = src < HALF

    # per-(core, local node) in-degrees for each stream
    key = core * NPC + loc
    lo_deg = np.bincount(key[is_lo], minlength=NC * NPC).reshape(NC, NPC)
    hi_deg = np.bincount(key[~is_lo], minlength=NC * NPC).reshape(NC, NPC)
    tot_deg = lo_deg + hi_deg
    assert tot_deg.max() <= P, f"in-degree {tot_deg.max()} exceeds {P}"
    assert (lo_deg + hi_deg).reshape(-1).min() >= 0

    # balance nodes into tiles; then KB = max blocks over all (core, tile)
    kb_lo_t = max(1, int(math.ceil(lo_deg.sum(1).max() / (NT * P))))
    kb_hi_t = max(1, int(math.ceil(hi_deg.sum(1).max() / (NT * P))))
    slots = np.zeros((NC, NPC), np.int64)
    KB_lo = kb_lo_t
    KB_hi = kb_hi_t
    for c in range(NC):
        s, bl, bh = _balance_tiles(lo_deg[c], hi_deg[c], kb_lo_t, kb_hi_t)
        slots[c] = s
        KB_lo = max(KB_lo, int(math.ceil(bl.max() / P)))
        KB_hi = max(KB_hi, int(math.ceil(bh.max() / P)))
    KBT = KB_lo + KB_hi
    NBLK = NT * KBT
    NI_lo = NT * KB_lo * P
    NI_hi = NT * KB_hi * P

    # per-edge slot position
    eslot = slots[core, loc]          # global slot id of dst
    tl = eslot >> 7                   # tile
    lane = eslot & 127                # node lane within tile

    # host edge-softmax (same math as the reference, shift-free exp since
    # logits are O(1)): aw_e = exp(l_e) / sum_{e': dst=dst_e} exp(l_e')
    ex_h = np.exp(logit.astype(np.float64))
    esum = np.bincount(dst, weights=ex_h, minlength=N_NODES)
    aw_e = (ex_h / esum[dst]).astype(np.float32)

    aw_arr = np.zeros((NC, P, NBLK), np.float32)
    dl_arr = np.zeros((NC, P, NBLK), np.float32)
    # Pad slots hold idx -1: the gather ucode trims trailing negatives at
    # runtime (per core), skipping their descriptor generation AND the DMA.
    # Skipped slots keep stale SBUF data, harmless since their one-hot
    # column is all-zero -- except on a buffer's FIRST use (first 6 groups,
    # uninitialized SBUF could be NaN as bf16), where pads gather row 0.
    ilo_flat = np.full((NC, NI_lo), -1, np.int16)
    ihi_flat = np.full((NC, NI_hi), -1, np.int16)

    tkey = core * NT + tl
    for stream, KB, ifl, coff in ((is_lo, KB_lo, ilo_flat, 0),
                                  (~is_lo, KB_hi, ihi_flat, KB_lo)):
        sel = np.flatnonzero(stream)
        order = sel[np.argsort(tkey[sel], kind="stable")]
        cnts = np.bincount(tkey[sel], minlength=NC * NT)
        assert cnts.max() <= KB * P
        starts = np.zeros(NC * NT, np.int64)
        starts[1:] = np.cumsum(cnts)[:-1]
        rank = np.arange(order.size, dtype=np.int64) - starts[tkey[order]]
        j = rank >> 7
        p = rank & 127
        ce = core[order]
        te = tl[order]
        col = te * KBT + coff + j
        aw_arr[ce, p, col] = aw_e[order]
        dl_arr[ce, p, col] = lane[order].astype(np.float32)
        sv = src[order] - (0 if coff == 0 else HALF)
        ifl[ce, (te * KB + j) * P + p] = sv.astype(np.int16)
        # first-use buffers (first GT*8 tiles, ring of 8): pads gather row 0
        first = 8 * _GT * KB * P
        ifl[:, :first][ifl[:, :first] < 0] = 0

    def wrap16(flat):  # [NC, L] -> [NC, 128, L//16]
        L = flat.shape[1]
        w = flat.reshape(NC, L // 16, 16).transpose(0, 2, 1)  # [NC,16,L//16]
        return np.ascontiguousarray(np.tile(w, (1, 8, 1)))

    ilo = wrap16(ilo_flat)
    ihi = wrap16(ihi_flat)

    nfT = np.zeros((NC, P, NTP), bf16)
    inv_slots = np.zeros((NC, NPC), np.int64)
    for c in range(NC):
        nfT[c][:, slots[c]] = nf[c * NPC : (c + 1) * NPC].T
        inv_slots[c] = slots[c]

    shared = {
        "nf": nf if _F32G else nf.astype(bf16),
        "w_projT": np.ascontiguousarray(np.asarray(W_proj, np.float32).T).astype(bf16),
        "w_ihT": np.ascontiguousarray(np.asarray(W_ih, np.float32).T).astype(bf16),
        "w_hhT": np.ascontiguousarray(np.asarray(W_hh, np.float32).T).astype(bf16),
        "b_projc": np.asarray(b_proj, np.float32).reshape(D, 1),
        "b_ih3": np.ascontiguousarray(np.asarray(b_ih, np.float32).reshape(3, D).T),
        "b_hh3": np.ascontiguousarray(np.asarray(b_hh, np.float32).reshape(3, D).T),
        "iota": np.ascontiguousarray(
            np.broadcast_to(np.arange(P, dtype=np.float32), (P, P))),
        "iotab": np.ascontiguousarray(
            np.broadcast_to(np.arange(P, dtype=np.float32), (P, P))).astype(bf16),
    }
    in_maps = []
    for c in range(NC):
        m = dict(shared)
        m["idx_lo"] = ilo[c]
        m["idx_hi"] = ihi[c]
        m["aw"] = aw_arr[c]
        m["awb"] = aw_arr[c].astype(bf16)
        m["dstlocb"] = dl_arr[c].astype(bf16)
        m["dstneg"] = -dl_arr[c]
        m["nfT"] = nfT[c]
        in_maps.append(m)
    return in_maps, KB_lo, KB_hi, inv_slots


def _run(inputs, trace=False):
    from concourse.bass_utils import run_bass_kernel_spmd

    in_maps, KB_lo, KB_hi, slots = _prep_inputs(**inputs)
    key = (KB_lo, KB_hi, _SP, _GT, _F32G, _NQ, _AMOD)
    if key not in _nc_cache:
        _nc_cache[key] = _build_nc(KB_lo, KB_hi)
    nc = _nc_cache[key]
    res = run_bass_kernel_spmd(nc, in_maps, core_ids=list(range(NC)), trace=trace)
    out = np.empty((N_NODES, D), np.float32)
    for c in range(NC):
        hT = res.results[c]["hT"]
        out[c * NPC : (c + 1) * NPC] = hT[:, slots[c]].T
    return out, res


def kernel(**inputs):
    out, _ = _run(inputs, trace=False)
    return out



# revision 46
# speedup vs baseline: 1.0203x; 1.0203x over previous
"""AttentiveGRU2 message-passing kernel for 8 Trainium2 NeuronCores.

Math (equivalent to the reference; projection moved after aggregation,
edge-softmax weights aw_e = exp(l_e)/esum[dst_e] computed in host prep
alongside the index layouts):

    uT[f,n] = sum_{e: dst=n} aw_e * f_{src_e}[f]   (PE one-hot matmuls)
    cT      = W_proj @ uT;  ctx0 = elu(cT + b_proj) + 1
    GRU(ctx, f) -> relu(h')     (ctx+1 shift folded into gate biases)

Sharding: nodes are range-partitioned over 8 cores (6250 each); each core
owns its incoming edges.  Within a core, nodes are bin-packed (host-side
permutation) into 49 tiles of <=128 so per-tile edge counts are balanced.
Per tile, the segment-sum runs as PE matmuls in FEATURE-major orientation:
    aggT[f, n] += gathered[e, f]^T-contracted-with oh[e, n]
with oh[e, n] = ex_e * [dstloc_e == n] built on DVE in bf16.
Source rows are fetched with dma_gather (int16 indices; edges split into
src<32768 / src>=32768 streams gathered from two table bases) from a bf16
copy of node_feats.
"""

import math
import os

import numpy as np

_SP = int(os.environ.get("K_SP", "0"))     # dma_gather single_packet
_F32G = int(os.environ.get("K_F32G", "0"))  # gather fp32 rows (cheaper Q7/idx)
_GT = int(os.environ.get("K_GT", "3"))     # tiles per gather group
_NQ = int(os.environ.get("K_NQ", "4"))     # SWDGE queues (gather gen parallelism)
_AMOD = int(os.environ.get("K_AMOD", "4"))  # 1/AMOD of oh blocks take the ACT route

P = 128
N_NODES = 50000
N_EDGES = 800000
D = 128
NC = 8
NPC = N_NODES // NC          # 6250 nodes per core
NT = math.ceil(NPC / P)      # 49 node tiles per core
NTP = NT * P                 # 6272 padded node slots per core
HALF = 32768                 # int16 index split point

_nc_cache = {}


def _build_nc(KB_lo, KB_hi):
    import concourse.bacc as bacc
    import concourse.bass as bass
    import concourse.mybir as mybir
    import concourse.tile as tile

    f32 = mybir.dt.float32
    bf16 = mybir.dt.bfloat16
    i16 = mybir.dt.int16
    Alu = mybir.AluOpType
    Act = mybir.ActivationFunctionType

    GT = _GT
    KBT = KB_lo + KB_hi
    NBLK = NT * KBT
    NI_lo = NT * KB_lo * P       # total lo gather slots per core
    NI_hi = NT * KB_hi * P
    NG = math.ceil(NT / GT)      # gather groups / epilogue chunks

    nc = bacc.Bacc(None, target_bir_lowering=False, num_swdge_queues=_NQ)

    gdt = f32 if _F32G else bf16
    nf = nc.dram_tensor("nf", [N_NODES, D], gdt, kind="ExternalInput")
    idx_lo_d = nc.dram_tensor("idx_lo", [P, NI_lo // 16], i16, kind="ExternalInput")
    idx_hi_d = nc.dram_tensor("idx_hi", [P, NI_hi // 16], i16, kind="ExternalInput")
    aw_d = nc.dram_tensor("aw", [P, NBLK], f32, kind="ExternalInput")
    awb_d = nc.dram_tensor("awb", [P, NBLK], bf16, kind="ExternalInput")
    dstlocb_d = nc.dram_tensor("dstlocb", [P, NBLK], bf16, kind="ExternalInput")
    dstneg_d = nc.dram_tensor("dstneg", [P, NBLK], f32, kind="ExternalInput")
    nfT_d = nc.dram_tensor("nfT", [P, NTP], bf16, kind="ExternalInput")
    w_projT_d = nc.dram_tensor("w_projT", [D, D], bf16, kind="ExternalInput")
    w_ihT_d = nc.dram_tensor("w_ihT", [D, 3 * D], bf16, kind="ExternalInput")
    w_hhT_d = nc.dram_tensor("w_hhT", [D, 3 * D], bf16, kind="ExternalInput")
    b_projc_d = nc.dram_tensor("b_projc", [D, 1], f32, kind="ExternalInput")
    b_ih3_d = nc.dram_tensor("b_ih3", [D, 3], f32, kind="ExternalInput")
    b_hh3_d = nc.dram_tensor("b_hh3", [D, 3], f32, kind="ExternalInput")
    iota_d = nc.dram_tensor("iota", [P, P], f32, kind="ExternalInput")
    iotab_d = nc.dram_tensor("iotab", [P, P], bf16, kind="ExternalInput")
    hT_d = nc.dram_tensor("hT", [P, NTP], f32, kind="ExternalOutput")

    with tile.TileContext(nc) as tc:
        with (
            tc.tile_pool(name="const", bufs=1) as cp,
            tc.tile_pool(name="glo_p", bufs=6) as glo_p,
            tc.tile_pool(name="ghi_p", bufs=6) as ghi_p,
            tc.tile_pool(name="oh_p", bufs=10) as oh_p,
            tc.tile_pool(name="u_p", bufs=3) as u_p,
            tc.tile_pool(name="wk", bufs=2) as wk,
            tc.tile_pool(name="ps_agg", bufs=2, space="PSUM") as ps_agg,
            tc.tile_pool(name="ps_ct", bufs=1, space="PSUM") as ps_ct,
            tc.tile_pool(name="ps_misc", bufs=1, space="PSUM") as ps_misc,
            tc.tile_pool(name="ps_g", bufs=1, space="PSUM") as ps_g,
        ):
            # ---- resident tiles -------------------------------------------
            # idx tables split into a small head (first 2 groups) + tail so
            # the first gathers only wait for a tiny DMA, not the whole table
            LOH = 2 * GT * KB_lo * P // 16   # head cols (idx/16)
            HIH = 2 * GT * KB_hi * P // 16
            idx_lo_a = cp.tile([P, LOH], i16)
            nc.sync.dma_start(out=idx_lo_a[:], in_=idx_lo_d[:, :LOH])
            idx_hi_a = cp.tile([P, HIH], i16)
            nc.sync.dma_start(out=idx_hi_a[:], in_=idx_hi_d[:, :HIH])
            idx_lo_b = cp.tile([P, NI_lo // 16 - LOH], i16)
            nc.sync.dma_start(out=idx_lo_b[:], in_=idx_lo_d[:, LOH:])
            idx_hi_b = cp.tile([P, NI_hi // 16 - HIH], i16)
            nc.sync.dma_start(out=idx_hi_b[:], in_=idx_hi_d[:, HIH:])
            aw = cp.tile([P, NBLK], f32)
            nc.sync.dma_start(out=aw[:], in_=aw_d[:])
            awb = cp.tile([P, NBLK], bf16)
            nc.sync.dma_start(out=awb[:], in_=awb_d[:])
            dstlocb = cp.tile([P, NBLK], bf16)
            nc.sync.dma_start(out=dstlocb[:], in_=dstlocb_d[:])
            dstneg = cp.tile([P, NBLK], f32)
            nc.sync.dma_start(out=dstneg[:], in_=dstneg_d[:])
            nfT = cp.tile([P, NTP], bf16)
            nc.sync.dma_start(out=nfT[:], in_=nfT_d[:])
            w_projT = cp.tile([D, D], bf16)
            nc.sync.dma_start(out=w_projT[:], in_=w_projT_d[:])
            w_ihT = cp.tile([D, 3 * D], bf16)
            nc.sync.dma_start(out=w_ihT[:], in_=w_ihT_d[:])
            w_hhT = cp.tile([D, 3 * D], bf16)
            nc.sync.dma_start(out=w_hhT[:], in_=w_hhT_d[:])
            b_projc = cp.tile([D, 1], f32)
            nc.sync.dma_start(out=b_projc[:], in_=b_projc_d[:])
            b_ih3 = cp.tile([D, 3], f32)
            nc.sync.dma_start(out=b_ih3[:], in_=b_ih3_d[:])
            b_hh3 = cp.tile([D, 3], f32)
            nc.sync.dma_start(out=b_hh3[:], in_=b_hh3_d[:])
            iota = cp.tile([P, P], f32)
            nc.sync.dma_start(out=iota[:], in_=iota_d[:])
            iotab = cp.tile([P, P], bf16)
            nc.sync.dma_start(out=iotab[:], in_=iotab_d[:])

            ones_c = cp.tile([P, 1], bf16)
            nc.vector.memset(ones_c[:], 1.0)

            # Warmup gather: pays the gather ucode's one-time ~6us Q7 IRAM
            # load while the real idx tables are still streaming in, so the
            # first real gather starts hot.  Depends only on the memset.
            idxw = cp.tile([P, 8], i16)
            nc.vector.memset(idxw[:], 0)
            gw = cp.tile([P, P], gdt)
            nc.gpsimd.dma_gather(
                out_ap=gw[:, :P].rearrange("p (n e) -> p n e", e=P),
                in_ap=nf[:],
                idxs_ap=idxw[:],
                num_idxs=P,
                num_idxs_reg=P,
                elem_size=P,
                single_packet=bool(_SP),
                queue_num=0,
            )

            # GRU input biases, adjusted for the ctx+1 shift:
            #   ctx0 = elu(c) + 1, so b_ih_g' = b_ih_g - rowsum(W_ih_g).
            rs = ps_misc.tile([P, 3], f32, name="rs", tag="misc")
            for g in range(3):
                nc.tensor.matmul(
                    rs[:, g : g + 1],
                    lhsT=w_ihT[:, g * D : (g + 1) * D],
                    rhs=ones_c[:],
                    start=(g == 0),
                    stop=(g == 2),
                    skip_group_check=True,
                )
            bb = cp.tile([P, 3], f32)
            nc.vector.tensor_add(bb[:, 0:2], b_ih3[:, 0:2], b_hh3[:, 0:2])
            nc.vector.tensor_copy(bb[:, 2:3], b_ih3[:, 2:3])
            nc.vector.tensor_sub(bb[:], bb[:], rs[:])

            # ---- main loop -------------------------------------------------
            # num_idxs registers hoisted out of the loop (one reg per distinct
            # count) so the Pool instruction stream between gathers stays
            # short and the exec queue can run gathers ahead.
            reg_cache = {}

            def idx_reg(n):
                if n not in reg_cache:
                    reg_cache[n] = nc.gpsimd.to_reg(n)
                return reg_cache[n]

            # Each (stream, group) gather is split in two halves; every piece
            # goes to the least-loaded SWDGE queue so the four Q7 core pairs
            # share descriptor generation evenly (lo pieces are ~2x hi ones).
            qloads = [0] * _NQ

            def emit_gather(buf, src_ap, idx_tile, c0_idx, n):
                # buf columns [0, n) filled from idx columns starting c0_idx
                h1 = (n // 2 // P) * P
                for (off, cnt) in ((0, h1), (h1, n - h1)):
                    if cnt == 0:
                        continue
                    q = min(range(_NQ), key=lambda i: qloads[i])
                    qloads[q] += cnt
                    nc.gpsimd.dma_gather(
                        out_ap=buf[:, off : off + cnt].rearrange(
                            "p (n e) -> p n e", e=P),
                        in_ap=src_ap,
                        idxs_ap=idx_tile[:, (c0_idx + off) // 16 :
                                         (c0_idx + off + cnt) // 16],
                        num_idxs=cnt,
                        num_idxs_reg=idx_reg(cnt),
                        elem_size=P,
                        single_packet=bool(_SP),
                        queue_num=q,
                    )

            for g in range(NG):
                t0 = g * GT
                t1 = min(t0 + GT, NT)
                nt = t1 - t0
                n_lo = nt * KB_lo * P
                n_hi = nt * KB_hi * P

                lo_head = 2 * GT * KB_lo * P
                hi_head = 2 * GT * KB_hi * P
                glo = glo_p.tile([P, GT * KB_lo * P], gdt, name="glo")
                if g < 2:
                    emit_gather(glo, nf[:], idx_lo_a, t0 * KB_lo * P, n_lo)
                else:
                    emit_gather(glo, nf[:], idx_lo_b,
                                t0 * KB_lo * P - lo_head, n_lo)
                ghi = ghi_p.tile([P, GT * KB_hi * P], gdt, name="ghi")
                if g < 2:
                    emit_gather(ghi, nf[HALF:, :], idx_hi_a, t0 * KB_hi * P, n_hi)
                else:
                    emit_gather(ghi, nf[HALF:, :], idx_hi_b,
                                t0 * KB_hi * P - hi_head, n_hi)

                uch = u_p.tile([P, GT * P], bf16, name="uch")
                for t in range(t0, t1):
                    ps = ps_agg.tile([P, P], f32, name="ps")
                    for j in range(KBT):
                        col = t * KBT + j
                        if j < KB_lo:
                            lhs = glo[:, ((t - t0) * KB_lo + j) * P :][:, :P]
                        else:
                            jj = j - KB_lo
                            lhs = ghi[:, ((t - t0) * KB_hi + jj) * P :][:, :P]
                        oh = oh_p.tile([P, P], gdt, name="oh")
                        if (t * KBT + j) % _AMOD == _AMOD - 1:
                            # ACT route (bf16 for the 2x perf mode):
                            #   oh = relu(aw - 1e6*(iota - dst)^2)
                            # (iota-dst)^2 <= 127^2 rounds in bf16 but stays
                            # nonzero, which is all the -1e6 mask needs.
                            sq = oh_p.tile([P, P], bf16, name="sq", tag="sq")
                            nc.scalar.activation(sq[:], iotab[:], Act.Square,
                                                 bias=dstneg[:, col : col + 1])
                            nc.scalar.activation(oh[:], sq[:], Act.Relu,
                                                 scale=-1e6,
                                                 bias=aw[:, col : col + 1])
                        else:
                            # DVE route, one bf16 STT op with the weight
                            # column broadcast (0-stride) along free:
                            #   oh = (iota == dst) * aw_bc
                            nc.vector.scalar_tensor_tensor(
                                oh[:], iotab[:],
                                dstlocb[:, col : col + 1],
                                awb[:, col : col + 1].to_broadcast((P, P)),
                                Alu.is_equal, Alu.mult,
                            )
                        nc.tensor.matmul(
                            ps[:, 0:P], lhsT=lhs, rhs=oh[:],
                            start=(j == 0), stop=(j == KBT - 1),
                            skip_group_check=True,
                        )
                    # softmax weights are pre-normalized; just move the
                    # accumulated tile PSUM -> SBUF for the projection GEMM
                    nc.vector.tensor_copy(
                        uch[:, (t - t0) * P : (t - t0 + 1) * P], ps[:, 0:P],
                    )

                # ---- chunk epilogue (nodes t0*P .. t1*P) ------------------
                w = nt * P
                fch = nfT[:, t0 * P : t0 * P + w]

                ct = ps_ct.tile([P, GT * P], f32, name="ct", tag="ct")
                nc.tensor.matmul(ct[:, :w], lhsT=w_projT[:], rhs=uch[:, :w],
                                 start=True, stop=True, skip_group_check=True)
                # ctx0 = elu(c + b_proj) + 1 = relu(c+b) + min(exp(c+b), 1)
                expc = wk.tile([P, GT * P], bf16, name="expc")
                nc.scalar.activation(expc[:, :w], ct[:, :w], Act.Exp,
                                     bias=b_projc[:, 0:1])
                reluc = wk.tile([P, GT * P], bf16, name="reluc")
                nc.scalar.activation(reluc[:, :w], ct[:, :w], Act.Relu,
                                     bias=b_projc[:, 0:1])
                ctx0 = wk.tile([P, GT * P], bf16, name="ctx0")
                nc.vector.scalar_tensor_tensor(
                    ctx0[:, :w], expc[:, :w], 1.0, reluc[:, :w],
                    Alu.min, Alu.add,
                )

                # GRU gates r | z | in_ | hn.  For GT<=2 two gates pack per
                # 2KB PSUM bank and the bank-zeroing start=True of the pair's
                # first matmul covers both; for wider GT each gate gets its
                # own bank-aligned 512-f32 stripe and zeroes it itself.
                if GT * P <= 256:
                    GS = GT * P
                    zs, hs = False, False   # z/hn rely on pair-mate's zeroing
                else:
                    GS = 512
                    zs, hs = True, True
                pg = ps_g.tile([P, 4 * GS], f32, name="pg")
                nc.tensor.matmul(pg[:, 0 * GS :][:, :w], lhsT=w_ihT[:, 0:D],
                                 rhs=ctx0[:, :w], start=True, stop=False,
                                 skip_group_check=True)
                nc.tensor.matmul(pg[:, 0 * GS :][:, :w], lhsT=w_hhT[:, 0:D],
                                 rhs=fch, start=False, stop=True,
                                 skip_group_check=True)
                nc.tensor.matmul(pg[:, 1 * GS :][:, :w], lhsT=w_ihT[:, D : 2 * D],
                                 rhs=ctx0[:, :w], start=zs, stop=False,
                                 skip_group_check=True)
                nc.tensor.matmul(pg[:, 1 * GS :][:, :w], lhsT=w_hhT[:, D : 2 * D],
                                 rhs=fch, start=False, stop=True,
                                 skip_group_check=True)
                nc.tensor.matmul(pg[:, 2 * GS :][:, :w], lhsT=w_ihT[:, 2 * D : 3 * D],
                                 rhs=ctx0[:, :w], start=True, stop=True,
                                 skip_group_check=True)
                nc.tensor.matmul(pg[:, 3 * GS :][:, :w], lhsT=w_hhT[:, 2 * D : 3 * D],
                                 rhs=fch, start=hs, stop=True,
                                 skip_group_check=True)

                rT = wk.tile([P, GT * P], f32, name="rT")
                nc.scalar.activation(rT[:, :w], pg[:, 0 * GS :][:, :w], Act.Sigmoid,
                                     bias=bb[:, 0:1])
                zT = wk.tile([P, GT * P], bf16, name="zT")
                nc.scalar.activation(zT[:, :w], pg[:, 1 * GS :][:, :w], Act.Sigmoid,
                                     bias=bb[:, 1:2])
                # rhn = (phn + b_hh_n) * r
                rhn = wk.tile([P, GT * P], f32, name="rhn")
                nc.vector.scalar_tensor_tensor(
                    rhn[:, :w], pg[:, 3 * GS :][:, :w], b_hh3[:, 2:3], rT[:, :w],
                    Alu.add, Alu.mult,
                )
                tn = wk.tile([P, GT * P], f32, name="tn")
                nc.vector.tensor_add(tn[:, :w], pg[:, 2 * GS :][:, :w], rhn[:, :w])
                nT = wk.tile([P, GT * P], bf16, name="nT")
                nc.scalar.activation(nT[:, :w], tn[:, :w], Act.Tanh,
                                     bias=bb[:, 2:3])
                # h = n + z*(f - n); out = relu(h)
                d2 = wk.tile([P, GT * P], bf16, name="d2")
                nc.vector.tensor_sub(d2[:, :w], fch, nT[:, :w])
                e2 = wk.tile([P, GT * P], bf16, name="e2")
                nc.vector.tensor_mul(e2[:, :w], zT[:, :w], d2[:, :w])
                h1 = wk.tile([P, GT * P], bf16, name="h1")
                nc.vector.tensor_add(h1[:, :w], nT[:, :w], e2[:, :w])
                hch = wk.tile([P, GT * P], f32, name="hch")
                nc.scalar.activation(hch[:, :w], h1[:, :w], Act.Relu)
                nc.sync.dma_start(out=hT_d[:, t0 * P : t0 * P + w],
                                  in_=hch[:, :w])

    nc.compile()
    return nc


def _balance_tiles(lo_deg, hi_deg, kb_lo_t, kb_hi_t):
    """Pack NPC nodes into NT bins of <=P nodes, balancing lo/hi edge sums.

    Returns slot[NPC] -> global slot id (tile*P + lane).  Greedy: nodes in
    descending total degree, placed in the feasible bin minimizing the max
    normalized load.
    """
    cap_lo = kb_lo_t * P
    cap_hi = kb_hi_t * P
    order = np.argsort(-(lo_deg + hi_deg), kind="stable")
    bin_lo = np.zeros(NT, np.int64)
    bin_hi = np.zeros(NT, np.int64)
    bin_cnt = np.zeros(NT, np.int64)
    slot = np.zeros(NPC, np.int64)
    for v in order:
        load = np.maximum((bin_lo + lo_deg[v]) / cap_lo,
                          (bin_hi + hi_deg[v]) / cap_hi)
        load[bin_cnt >= P] = np.inf
        b = int(np.argmin(load))
        slot[v] = b * P + bin_cnt[b]
        bin_cnt[b] += 1
        bin_lo[b] += lo_deg[v]
        bin_hi[b] += hi_deg[v]
    return slot, bin_lo, bin_hi


def _prep_inputs(edge_logits, node_feats, src, dst, W_proj, b_proj, W_ih, b_ih,
                 W_hh, b_hh):
    """Host-side index preprocessing + layout. Returns (in_maps, KB_lo, KB_hi, slots)."""
    import ml_dtypes

    bf16 = ml_dtypes.bfloat16
    src = np.asarray(src).astype(np.int64)
    dst = np.asarray(dst).astype(np.int64)
    logit = np.asarray(edge_logits, dtype=np.float32).reshape(-1)
    nf = np.ascontiguousarray(np.asarray(node_feats, dtype=np.float32))

    core = dst // NPC
    loc = dst - core * NPC
    is_lo = src < HALF

    # per-(core, local node) in-degrees for each stream
    key = core * NPC + loc
    lo_deg = np.bincount(key[is_lo], minlength=NC * NPC).reshape(NC, NPC)
    hi_deg = np.bincount(key[~is_lo], minlength=NC * NPC).reshape(NC, NPC)
    tot_deg = lo_deg + hi_deg
    assert tot_deg.max() <= P, f"in-degree {tot_deg.max()} exceeds {P}"
    assert (lo_deg + hi_deg).reshape(-1).min() >= 0

    # balance nodes into tiles; then KB = max blocks over all (core, tile)
    kb_lo_t = max(1, int(math.ceil(lo_deg.sum(1).max() / (NT * P))))
    kb_hi_t = max(1, int(math.ceil(hi_deg.sum(1).max() / (NT * P))))
    slots = np.zeros((NC, NPC), np.int64)
    KB_lo = kb_lo_t
    KB_hi = kb_hi_t
    for c in range(NC):
        s, bl, bh = _balance_tiles(lo_deg[c], hi_deg[c], kb_lo_t, kb_hi_t)
        slots[c] = s
        KB_lo = max(KB_lo, int(math.ceil(bl.max() / P)))
        KB_hi = max(KB_hi, int(math.ceil(bh.max() / P)))
    KBT = KB_lo + KB_hi
    NBLK = NT * KBT
    NI_lo = NT * KB_lo * P
    NI_hi = NT * KB_hi * P

    # per-edge slot position
    eslot = slots[core, loc]          # global slot id of dst
    tl = eslot >> 7                   # tile
    lane = eslot & 127                # node lane within tile

    # host edge-softmax (same math as the reference, shift-free exp since
    # logits are O(1)): aw_e = exp(l_e) / sum_{e': dst=dst_e} exp(l_e')
    ex_h = np.exp(logit.astype(np.float64))
    esum = np.bincount(dst, weights=ex_h, minlength=N_NODES)
    aw_e = (ex_h / esum[dst]).astype(np.float32)

    aw_arr = np.zeros((NC, P, NBLK), np.float32)
    dl_arr = np.zeros((NC, P, NBLK), np.float32)
    # Pad slots hold idx -1: the gather ucode trims trailing negatives at
    # runtime (per core), skipping their descriptor generation AND the DMA.
    # Skipped slots keep stale SBUF data, harmless since their one-hot
    # column is all-zero -- except on a buffer's FIRST use (first 6 groups,
    # uninitialized SBUF could be NaN as bf16), where pads gather row 0.
    ilo_flat = np.full((NC, NI_lo), -1, np.int16)
    ihi_flat = np.full((NC, NI_hi), -1, np.int16)

    tkey = core * NT + tl
    for stream, KB, ifl, coff in ((is_lo, KB_lo, ilo_flat, 0),
                                  (~is_lo, KB_hi, ihi_flat, KB_lo)):
        sel = np.flatnonzero(stream)
        order = sel[np.argsort(tkey[sel], kind="stable")]
        cnts = np.bincount(tkey[sel], minlength=NC * NT)
        assert cnts.max() <= KB * P
        starts = np.zeros(NC * NT, np.int64)
        starts[1:] = np.cumsum(cnts)[:-1]
        rank = np.arange(order.size, dtype=np.int64) - starts[tkey[order]]
        j = rank >> 7
        p = rank & 127
        ce = core[order]
        te = tl[order]
        col = te * KBT + coff + j
        aw_arr[ce, p, col] = aw_e[order]
        dl_arr[ce, p, col] = lane[order].astype(np.float32)
        sv = src[order] - (0 if coff == 0 else HALF)
        ifl[ce, (te * KB + j) * P + p] = sv.astype(np.int16)
        # first-use buffers (first GT*8 tiles, ring of 8): pads gather row 0
        first = 8 * _GT * KB * P
        ifl[:, :first][ifl[:, :first] < 0] = 0

    def wrap16(flat):  # [NC, L] -> [NC, 128, L//16]
        L = flat.shape[1]
        w = flat.reshape(NC, L // 16, 16).transpose(0, 2, 1)  # [NC,16,L//16]
        return np.ascontiguousarray(np.tile(w, (1, 8, 1)))

    ilo = wrap16(ilo_flat)
    ihi = wrap16(ihi_flat)

    nfT = np.zeros((NC, P, NTP), bf16)
    inv_slots = np.zeros((NC, NPC), np.int64)
    for c in range(NC):
        nfT[c][:, slots[c]] = nf[c * NPC : (c + 1) * NPC].T
        inv_slots[c] = slots[c]

    shared = {
        "nf": nf if _F32G else nf.astype(bf16),
        "w_projT": np.ascontiguousarray(np.asarray(W_proj, np.float32).T).astype(bf16),
        "w_ihT": np.ascontiguousarray(np.asarray(W_ih, np.float32).T).astype(bf16),
        "w_hhT": np.ascontiguousarray(np.asarray(W_hh, np.float32).T).astype(bf16),
        "b_projc": np.asarray(b_proj, np.float32).reshape(D, 1),
        "b_ih3": np.ascontiguousarray(np.asarray(b_ih, np.float32).reshape(3, D).T),
        "b_hh3": np.ascontiguousarray(np.asarray(b_hh, np.float32).reshape(3, D).T),
        "iota": np.ascontiguousarray(
            np.broadcast_to(np.arange(P, dtype=np.float32), (P, P))),
        "iotab": np.ascontiguousarray(
            np.broadcast_to(np.arange(P, dtype=np.float32), (P, P))).astype(bf16),
    }
    in_maps = []
    for c in range(NC):
        m = dict(shared)
        m["idx_lo"] = ilo[c]
        m["idx_hi"] = ihi[c]
        m["aw"] = aw_arr[c]
        m["awb"] = aw_arr[c].astype(bf16)
        m["dstlocb"] = dl_arr[c].astype(bf16)
        m["dstneg"] = -dl_arr[c]
        m["nfT"] = nfT[c]
        in_maps.append(m)
    return in_maps, KB_lo, KB_hi, inv_slots


def _run(inputs, trace=False):
    from concourse.bass_utils import run_bass_kernel_spmd

    in_maps, KB_lo, KB_hi, slots = _prep_inputs(**inputs)
    key = (KB_lo, KB_hi, _SP, _GT, _F32G, _NQ, _AMOD)
    if key not in _nc_cache:
        _nc_cache[key] = _build_nc(KB_lo, KB_hi)
    nc = _nc_cache[key]
    res = run_bass_kernel_spmd(nc, in_maps, core_ids=list(range(NC)), trace=trace)
    out = np.empty((N_NODES, D), np.float32)
    for c in range(NC):
        hT = res.results[c]["hT"]
        out[c * NPC : (c + 1) * NPC] = hT[:, slots[c]].T
    return out, res


def kernel(**inputs):
    out, _ = _run(inputs, trace=False)
    return out

